# revision 15
# baseline (speedup 1.0000x reference)
"""Expert-parallel CMoE kernel for 8 Trainium2 NeuronCores.

Sharding (hardcoded for B=8, T=2048, D=1024, F=2048, E=16, C=1024):
  core k owns batch k (token shift, receptance, output) and experts
  {2k, 2k+1} (FFN). Hash routing is int math on token_ids, done on host;
  the resulting permutations ship to the cores as index tensors.

v4 schedule: x arrives bf16 and loads in four 1MB DMAs; the token shift
runs on the PE as dxprev = (S - I)@x + E@x_prev (exact in fp32 PSUM), so
phase A needs no second x stream; xr transposes on-chip via the XBAR
DMA; the receptance matmuls fill the PE while the two dispatch
all-to-alls fly; expert weights stream in 1MB groups just-in-time; the
combine is four balanced all-to-alls (expert x slot-parity) overlapped
with FFN compute; phase D runs in 512-token steps. DMA instruction
count is kept low throughout so the tile framework's DMA-completion
semaphores don't chain independent streams behind the collectives.
"""
import sys

for _p in ("/opt/trn_rl_repo", "/root/.axon_site/_ro/trn_rl_repo"):
    if _p not in sys.path:
        sys.path.append(_p)

import numpy as np
import ml_dtypes

import concourse.bass as bass
import concourse.bacc as bacc
import concourse.mybir as mybir
import concourse.tile as tile
from concourse.bass_utils import run_bass_kernel_spmd

P = 128
B, T, D, F, E = 8, 2048, 1024, 2048, 16
N = B * T
C = max(4, N // E)          # 1024
HASH_PRIME = 5099
NCORES = 8
EPC = E // NCORES           # experts per core = 2
DC = D // P                 # 8
FC = F // P                 # 16
TP = T // P                 # 16
HC = C // 2                 # 512, combine half size
BF16 = mybir.dt.bfloat16
F32 = mybir.dt.float32
I16 = mybir.dt.int16
I32 = mybir.dt.int32
nbf16 = ml_dtypes.bfloat16
AF = mybir.ActivationFunctionType

_CACHE = {}


def _r16(v):
    return int(-(-int(v) // 16) * 16)


def _wrap16(a):
    a = np.asarray(a, np.int16)
    w = a.reshape(-1, 16).T.copy()       # j at [j%16, j//16]
    return np.tile(w, (8, 1))            # replicated across 8 Q7 cores


def _route(token_ids):
    tid = np.asarray(token_ids).reshape(N).astype(np.int64)
    e = (tid * HASH_PRIME) % E
    onehot = (e[:, None] == np.arange(E)).astype(np.int64)
    pos = onehot.cumsum(0)[np.arange(N), e] - 1
    keep = pos < C
    return e, pos, keep


def _build_indices(token_ids):
    e, pos, keep = _route(token_ids)
    src = np.arange(N) // T
    dst = e // EPC
    el = e % EPC
    local_t = np.arange(N) % T
    # expert slots interleaved by arrival parity so each (src, dst) pair
    # contributes evenly to both combine halves
    slot = (pos % 2) * HC + pos // 2
    h = pos % 2

    def pack(mask):
        rank = np.zeros(N, np.int64)
        cnt = np.zeros((NCORES, NCORES), np.int64)
        for n in np.nonzero(mask)[0]:
            rank[n] = cnt[src[n], dst[n]]
            cnt[src[n], dst[n]] += 1
        return rank, _r16(cnt.max())

    # ---- dispatch: two chunks split by local token index
    in_a = keep & (local_t < T // 2)
    in_b = keep & (local_t >= T // 2)
    rank_a, Ka = pack(in_a)
    rank_b, Kb = pack(in_b)
    srcA = np.where(in_a, dst * Ka + rank_a, NCORES * Ka)
    srcB = np.where(in_b, dst * Kb + rank_b, NCORES * Kb)

    ZR1 = NCORES * (Ka + Kb)
    recv_row = np.full((NCORES, EPC * C), ZR1, np.int64)
    for n in np.nonzero(in_a)[0]:
        recv_row[dst[n], el[n] * C + slot[n]] = src[n] * Ka + rank_a[n]
    for n in np.nonzero(in_b)[0]:
        recv_row[dst[n], el[n] * C + slot[n]] = \
            NCORES * Ka + src[n] * Kb + rank_b[n]

    # ---- combine: four chunks (el, h)
    Kc = {}
    rank_c = np.zeros(N, np.int64)
    for eli in range(EPC):
        for hi in range(2):
            m = keep & (el == eli) & (h == hi)
            r_, K_ = pack(m)
            Kc[(eli, hi)] = K_
            rank_c[m] = r_[m]
    chunks = [(0, 0), (0, 1), (1, 0), (1, 1)]
    off = {}
    acc = 0
    for ch in chunks:
        off[ch] = acc
        acc += NCORES * Kc[ch]
    ZROW = acc

    s2 = np.zeros((NCORES, EPC, C), np.int64)
    for eli in range(EPC):
        for hi in range(2):
            s2[:, eli, hi * HC:(hi + 1) * HC] = NCORES * Kc[(eli, hi)]
    ygather = np.full(N, ZROW, np.int64)
    for n in np.nonzero(keep)[0]:
        ch = (el[n], h[n])
        s2[dst[n], el[n], slot[n]] = src[n] * Kc[ch] + rank_c[n]
        ygather[n] = off[ch] + dst[n] * Kc[ch] + rank_c[n]

    per_core = []
    for k in range(NCORES):
        tok = slice(k * T, (k + 1) * T)
        per_core.append({
            "srcA32": srcA[tok].astype(np.int32).reshape(TP, P).T.copy(),
            "srcB32": srcB[tok].astype(np.int32).reshape(TP, P).T.copy(),
            "slot16": _wrap16(recv_row[k]),
            "s232": np.ascontiguousarray(
                s2[k].reshape(EPC * C // P, P).T.astype(np.int32)),
            "ygather16": _wrap16(ygather[tok]),
        })
    cfg = (Ka, Kb, Kc[(0, 0)], Kc[(0, 1)], Kc[(1, 0)], Kc[(1, 1)])
    return cfg, per_core


def _build_nc(cfg):
    Ka, Kb, K00, K01, K10, K11 = cfg
    Kc = {(0, 0): K00, (0, 1): K01, (1, 0): K10, (1, 1): K11}
    chunks = [(0, 0), (0, 1), (1, 0), (1, 1)]
    off = {}
    acc = 0
    for ch in chunks:
        off[ch] = acc
        acc += NCORES * Kc[ch]
    R2 = acc
    ZR1 = NCORES * (Ka + Kb)
    rg = [list(range(NCORES))]

    nc = bacc.Bacc("TRN2", target_bir_lowering=False, debug=False,
                   num_devices=NCORES)

    x_ext = nc.dram_tensor("x_ext", [T + 1, D], BF16, kind="ExternalInput")
    maa_k = nc.dram_tensor("maa_k", [1, D], BF16, kind="ExternalInput")
    maa_r = nc.dram_tensor("maa_r", [1, D], BF16, kind="ExternalInput")
    wrt = nc.dram_tensor("wrt", [D, D], BF16, kind="ExternalInput")
    wk = nc.dram_tensor("wk", [EPC, D, F], BF16, kind="ExternalInput")
    wv = nc.dram_tensor("wv", [EPC, F, D], BF16, kind="ExternalInput")
    shiftM = nc.dram_tensor("shiftM", [P, P], BF16, kind="ExternalInput")
    eM = nc.dram_tensor("eM", [P, P], BF16, kind="ExternalInput")
    srcA32 = nc.dram_tensor("srcA32", [P, TP], I32, kind="ExternalInput")
    srcB32 = nc.dram_tensor("srcB32", [P, TP], I32, kind="ExternalInput")
    slot16 = nc.dram_tensor("slot16", [P, EPC * C // 16], I16,
                            kind="ExternalInput")
    s232 = nc.dram_tensor("s232", [P, EPC * C // P], I32,
                          kind="ExternalInput")
    ygather16 = nc.dram_tensor("ygather16", [P, T // 16], I16,
                               kind="ExternalInput")
    out = nc.dram_tensor("out", [T, D], F32, kind="ExternalOutput")

    with tile.TileContext(nc) as tc:
        with (
            tc.tile_pool(name="dram", bufs=1, space="DRAM") as dram,
            tc.tile_pool(name="misc", bufs=1) as misc,
        ):
            a1a = dram.tile([NCORES * Ka + 1, D], BF16)
            a1b = dram.tile([NCORES * Kb + 1, D], BF16)
            recv1 = dram.tile([ZR1 + 1, D], BF16)
            a2 = {ch: dram.tile([NCORES * Kc[ch] + 1, D], BF16,
                                name=f"a2_{ch[0]}{ch[1]}")
                  for ch in chunks}
            recv2 = dram.tile([R2 + 1, D], BF16)
            r_dram = dram.tile([T, D], BF16)

            sm_sb = misc.tile([P, P], BF16)
            nc.sync.dma_start(out=sm_sb[:], in_=shiftM[:])
            em_sb = misc.tile([P, P], BF16)
            nc.sync.dma_start(out=em_sb[:], in_=eM[:])
            maakb = misc.tile([P, D], BF16)
            nc.sync.dma_start(out=maakb[:], in_=maa_k[:].to_broadcast([P, D]))
            maarb = misc.tile([P, D], BF16)
            nc.sync.dma_start(out=maarb[:], in_=maa_r[:].to_broadcast([P, D]))

            # index tensors ride the gpsimd SWDGE (its queue is idle early)
            sA32 = misc.tile([P, TP], I32)
            nc.gpsimd.dma_start(out=sA32[:], in_=srcA32[:])
            sB32 = misc.tile([P, TP], I32)
            nc.gpsimd.dma_start(out=sB32[:], in_=srcB32[:])
            sl16 = misc.tile([P, EPC * C // 16], I16)
            nc.gpsimd.dma_start(out=sl16[:], in_=slot16[:])
            s2sb = misc.tile([P, EPC * C // P], I32)
            nc.gpsimd.dma_start(out=s2sb[:], in_=s232[:])
            yg16 = misc.tile([P, T // 16], I16)
            nc.gpsimd.dma_start(out=yg16[:], in_=ygather16[:])

            zrow = misc.tile([1, D], BF16)
            nc.vector.memzero(zrow[:])

            wrt_sb = misc.tile([P, DC, D], BF16)
            nc.scalar.dma_start(out=wrt_sb[:],
                                in_=wrt.rearrange("(c p) e -> p c e", p=P))

            wk_t = {}
            wv_t = {}
            with (
                tc.tile_pool(name="pwk", bufs=4) as pwk,
                tc.tile_pool(name="pwv", bufs=4) as pwv,
            ):
                # ---- phase A: PE token shift + dispatch scatter; then the
                # receptance matmuls run while the dispatch A2As fly.
                with (
                    tc.tile_pool(name="pa", bufs=4) as pa,
                    tc.tile_pool(name="pax", bufs=1) as pax,
                    tc.tile_pool(name="prx", bufs=1) as prx,
                    tc.tile_pool(name="prs", bufs=2) as prs,
                    tc.tile_pool(name="psB", bufs=2, space="PSUM") as psB,
                ):
                    xch = [pax.tile([P, 4, D], BF16, name=f"xch{q}")
                           for q in range(4)]
                    for q in range(4):
                        nc.sync.dma_start(
                            out=xch[q][:],
                            in_=x_ext[1 + q * 512:1 + (q + 1) * 512, :]
                            .rearrange("(a p) d -> p a d", p=P))
                    xm1 = pax.tile([P, D], BF16, name="xm1")
                    nc.vector.memzero(xm1[:])
                    nc.sync.dma_start(out=xm1[P - 1:P, :], in_=x_ext[0:1, :])
                    nc.sync.dma_start(out=recv1[ZR1:ZR1 + 1, :], in_=zrow[:])
                    nc.sync.dma_start(out=recv2[R2:R2 + 1, :], in_=zrow[:])
                    xrT = [prx.tile([P, DC, 4 * P], BF16, name=f"xrT{g}")
                           for g in range(4)]
                    xr_tiles = []

                    def emit_xbar(u):
                        nc.scalar.dma_start(
                            out=xrT[u // 4][:, :,
                                            (u % 4) * P:(u % 4 + 1) * P],
                            in_=xr_tiles[u][:], transpose=True)

                    def emit_recept_group(g):
                        rsb = prs.tile([P, 4, D], BF16, tag="rsb")
                        for u in range(4):
                            pr0 = psB.tile([P, 512], F32, space="PSUM",
                                           tag="pr0")
                            pr1 = psB.tile([P, 512], F32, space="PSUM",
                                           tag="pr1")
                            for dc in range(DC):
                                nc.tensor.matmul(
                                    out=pr0[:],
                                    lhsT=xrT[g][:, dc, u * P:(u + 1) * P],
                                    rhs=wrt_sb[:, dc, 0:512],
                                    start=(dc == 0), stop=(dc == DC - 1))
                                nc.tensor.matmul(
                                    out=pr1[:],
                                    lhsT=xrT[g][:, dc, u * P:(u + 1) * P],
                                    rhs=wrt_sb[:, dc, 512:1024],
                                    start=(dc == 0), stop=(dc == DC - 1))
                            nc.scalar.activation(out=rsb[:, u, 0:512],
                                                 in_=pr0[:], func=AF.Sigmoid)
                            nc.scalar.activation(out=rsb[:, u, 512:1024],
                                                 in_=pr1[:], func=AF.Sigmoid)
                        nc.scalar.dma_start(
                            out=r_dram[g * 512:(g + 1) * 512, :].rearrange(
                                "(a p) d -> p a d", p=P),
                            in_=rsb[:])

                    for t in range(TP):
                        xcb = xch[t // 4][:, t % 4, :]
                        prev = xm1[:] if t == 0 else \
                            xch[(t - 1) // 4][:, (t - 1) % 4, :]
                        psx0 = psB.tile([P, 512], F32, space="PSUM",
                                        tag="psx0")
                        psx1 = psB.tile([P, 512], F32, space="PSUM",
                                        tag="psx1")
                        # dxprev = (S - I) @ x_tile + E @ x_prev_tile
                        nc.tensor.matmul(out=psx0[:], lhsT=sm_sb[:],
                                         rhs=xcb[:, 0:512],
                                         start=True, stop=False)
                        nc.tensor.matmul(out=psx0[:], lhsT=em_sb[:],
                                         rhs=prev[:, 0:512],
                                         start=False, stop=True)
                        nc.tensor.matmul(out=psx1[:], lhsT=sm_sb[:],
                                         rhs=xcb[:, 512:1024],
                                         start=True, stop=False)
                        nc.tensor.matmul(out=psx1[:], lhsT=em_sb[:],
                                         rhs=prev[:, 512:1024],
                                         start=False, stop=True)
                        tk = pa.tile([P, D], BF16, tag="tk")
                        nc.vector.tensor_mul(out=tk[:, 0:512], in0=psx0[:],
                                             in1=maakb[:, 0:512])
                        nc.vector.tensor_mul(out=tk[:, 512:1024], in0=psx1[:],
                                             in1=maakb[:, 512:1024])
                        xk = pa.tile([P, D], BF16, tag="xk")
                        nc.vector.tensor_add(out=xk[:], in0=tk[:], in1=xcb)
                        if t < TP // 2:
                            nc.gpsimd.indirect_dma_start(
                                out=a1a[:],
                                out_offset=bass.IndirectOffsetOnAxis(
                                    ap=sA32[:, t:t + 1], axis=0),
                                in_=xk[:], in_offset=None)
                        else:
                            nc.gpsimd.indirect_dma_start(
                                out=a1b[:],
                                out_offset=bass.IndirectOffsetOnAxis(
                                    ap=sB32[:, t:t + 1], axis=0),
                                in_=xk[:], in_offset=None)
                        if t == TP // 2 - 1:
                            nc.gpsimd.collective_compute(
                                "AllToAll", mybir.AluOpType.bypass,
                                replica_groups=rg,
                                ins=[a1a[0:NCORES * Ka, :]],
                                outs=[recv1[0:NCORES * Ka, :]])
                        tr = pa.tile([P, D], BF16, tag="tr")
                        nc.vector.tensor_mul(out=tr[:, 0:512], in0=psx0[:],
                                             in1=maarb[:, 0:512])
                        nc.vector.tensor_mul(out=tr[:, 512:1024], in0=psx1[:],
                                             in1=maarb[:, 512:1024])
                        xr = pa.tile([P, D], BF16, tag="xr")
                        nc.vector.tensor_add(out=xr[:], in0=tr[:], in1=xcb)
                        xr_tiles.append(xr)
                        if t >= 2:
                            emit_xbar(t - 2)
                        if t % 4 == 0:
                            q = t // 4
                            wk_t[(0, q)] = pwk.tile([P, DC, 512], BF16,
                                                    tag="wk", name=f"wk0_{q}")
                            nc.scalar.dma_start(
                                out=wk_t[(0, q)][:],
                                in_=wk[0].rearrange("(c p) f -> p c f", p=P)
                                [:, :, q * 512:(q + 1) * 512])
                    nc.gpsimd.collective_compute(
                        "AllToAll", mybir.AluOpType.bypass, replica_groups=rg,
                        ins=[a1b[0:NCORES * Kb, :]],
                        outs=[recv1[NCORES * Ka:ZR1, :]])
                    emit_xbar(14)
                    emit_xbar(15)
                    for g in range(4):
                        emit_recept_group(g)

                # ---------------- phase C: expert FFNs
                with (
                    tc.tile_pool(name="pfx", bufs=2) as pfx,
                    tc.tile_pool(name="pfh", bufs=2) as pfh,
                    tc.tile_pool(name="pfr", bufs=2) as pfr,
                    tc.tile_pool(name="pfy", bufs=3) as pfy,
                    tc.tile_pool(name="psH", bufs=2, space="PSUM") as psH,
                    tc.tile_pool(name="psY", bufs=2, space="PSUM") as psY,
                ):
                    XTs = []
                    for el in range(EPC):
                        pair = []
                        for hh in range(2):
                            XTh = pfx.tile([P, DC, 512], BF16, tag=f"XT{hh}",
                                           name=f"XT{el}_{hh}")
                            col0 = el * (C // 16) + hh * 32
                            nc.gpsimd.dma_gather(
                                out_ap=XTh[:], in_ap=recv1[:],
                                idxs_ap=sl16[:, col0:col0 + 32],
                                num_idxs=512, num_idxs_reg=512, elem_size=D,
                                transpose=True)
                            pair.append(XTh)
                        XTs.append(pair)
                    for el in range(EPC):
                        XT0, XT1 = XTs[el]
                        ht = [pfh.tile([P, 4, C], BF16, tag=f"ht{g}",
                                       name=f"ht{el}_{g}")
                              for g in range(4)]
                        for ft in range(FC):
                            wkt = wk_t[(el, ft // 4)]
                            wcol = (ft % 4) * P
                            ph0 = psH.tile([P, 512], F32, space="PSUM",
                                           tag="ph0")
                            ph1 = psH.tile([P, 512], F32, space="PSUM",
                                           tag="ph1")
                            for dc in range(DC):
                                nc.tensor.matmul(
                                    out=ph0[:],
                                    lhsT=wkt[:, dc, wcol:wcol + P],
                                    rhs=XT0[:, dc, :],
                                    start=(dc == 0), stop=(dc == DC - 1))
                                nc.tensor.matmul(
                                    out=ph1[:],
                                    lhsT=wkt[:, dc, wcol:wcol + P],
                                    rhs=XT1[:, dc, :],
                                    start=(dc == 0), stop=(dc == DC - 1))
                            hr0 = pfr.tile([P, 512], BF16, tag="hr0")
                            nc.scalar.activation(out=hr0[:], in_=ph0[:],
                                                 func=AF.Relu)
                            nc.vector.tensor_mul(
                                out=ht[ft // 4][:, ft % 4, 0:512],
                                in0=hr0[:], in1=hr0[:])
                            hr1 = pfr.tile([P, 512], BF16, tag="hr1")
                            nc.scalar.activation(out=hr1[:], in_=ph1[:],
                                                 func=AF.Relu)
                            nc.vector.tensor_mul(
                                out=ht[ft // 4][:, ft % 4, 512:1024],
                                in0=hr1[:], in1=hr1[:])
                            if ft % 4 == 0:
                                # JIT weight prefetch on the scalar queue:
                                # wv for this expert's FFN2, wk for the next
                                # expert (after slot (0, q) frees)
                                q = ft // 4
                                wv_t[(el, q)] = pwv.tile(
                                    [P, 4, D], BF16, tag="wv",
                                    name=f"wv{el}_{q}")
                                nc.scalar.dma_start(
                                    out=wv_t[(el, q)][:],
                                    in_=wv[el][q * 512:(q + 1) * 512, :]
                                    .rearrange("(a p) d -> p a d", p=P))
                            if el == 0 and ft % 4 == 3:
                                q = ft // 4
                                wk_t[(1, q)] = pwk.tile([P, DC, 512], BF16,
                                                        tag="wk",
                                                        name=f"wk1_{q}")
                                nc.scalar.dma_start(
                                    out=wk_t[(1, q)][:],
                                    in_=wk[1].rearrange("(c p) f -> p c f",
                                                        p=P)
                                    [:, :, q * 512:(q + 1) * 512])
                        for tt in range(C // P):
                            py0 = psY.tile([P, 512], F32, space="PSUM",
                                           tag="py0")
                            py1 = psY.tile([P, 512], F32, space="PSUM",
                                           tag="py1")
                            for fc in range(FC):
                                wvt = wv_t[(el, fc // 4)]
                                nc.tensor.matmul(
                                    out=py0[:],
                                    lhsT=ht[fc // 4][:, fc % 4,
                                                     tt * P:(tt + 1) * P],
                                    rhs=wvt[:, fc % 4, 0:512],
                                    start=(fc == 0), stop=(fc == FC - 1))
                                nc.tensor.matmul(
                                    out=py1[:],
                                    lhsT=ht[fc // 4][:, fc % 4,
                                                     tt * P:(tt + 1) * P],
                                    rhs=wvt[:, fc % 4, 512:1024],
                                    start=(fc == 0), stop=(fc == FC - 1))
                            ysb = pfy.tile([P, D], BF16, tag="ysb")
                            nc.vector.tensor_copy(out=ysb[:, 0:512],
                                                  in_=py0[:])
                            nc.vector.tensor_copy(out=ysb[:, 512:1024],
                                                  in_=py1[:])
                            scol = el * (C // P) + tt
                            ch = (el, tt // 4)
                            nc.gpsimd.indirect_dma_start(
                                out=a2[ch][:],
                                out_offset=bass.IndirectOffsetOnAxis(
                                    ap=s2sb[:, scol:scol + 1], axis=0),
                                in_=ysb[:], in_offset=None)
                            if tt == 3 or tt == C // P - 1:
                                nc.gpsimd.collective_compute(
                                    "AllToAll", mybir.AluOpType.bypass,
                                    replica_groups=rg,
                                    ins=[a2[ch][0:NCORES * Kc[ch], :]],
                                    outs=[recv2[off[ch]:
                                                off[ch] + NCORES * Kc[ch], :]])

                # ---------------- phase D: gather own rows, multiply by r
                with tc.tile_pool(name="pd", bufs=3) as pd:
                    for st in range(T // 512):
                        yg = pd.tile([P, 4, D], BF16, tag="yg")
                        nc.gpsimd.dma_gather(
                            out_ap=yg[:], in_ap=recv2[:],
                            idxs_ap=yg16[:, st * 32:(st + 1) * 32],
                            num_idxs=512, num_idxs_reg=512, elem_size=D,
                            transpose=False)
                        rw = pd.tile([P, 4, D], BF16, tag="rw")
                        nc.sync.dma_start(
                            out=rw[:],
                            in_=r_dram[st * 512:(st + 1) * 512, :].rearrange(
                                "(a p) d -> p a d", p=P))
                        yo = pd.tile([P, 4, D], F32, tag="yo")
                        nc.vector.tensor_mul(out=yo[:], in0=yg[:], in1=rw[:])
                        nc.scalar.dma_start(
                            out=out[st * 512:(st + 1) * 512, :].rearrange(
                                "(a p) d -> p a d", p=P),
                            in_=yo[:])

    nc.finalize()
    return nc


def _prepare_inputs(x, token_ids, shift_state, time_maa_k, time_maa_r,
                    w_recept, w_key, w_value):
    cfg, idxs = _build_indices(token_ids)
    x = np.asarray(x, np.float32)
    shift = np.asarray(shift_state, np.float32)
    wrt = np.ascontiguousarray(np.asarray(w_recept, np.float32).T).astype(nbf16)
    wkb = np.asarray(w_key, np.float32).astype(nbf16)
    wvb = np.asarray(w_value, np.float32).astype(nbf16)
    mk = np.asarray(time_maa_k, np.float32)[None, :].astype(nbf16)
    mr = np.asarray(time_maa_r, np.float32)[None, :].astype(nbf16)
    # token-shift matrices: dxprev = (S - I) @ x_tile + E @ x_prev_tile
    sm = np.zeros((P, P), np.float32)
    for j in range(P):
        sm[j, j] = -1.0
        if j >= 1:
            sm[j - 1, j] = 1.0
    em = np.zeros((P, P), np.float32)
    em[P - 1, 0] = 1.0

    in_maps = []
    for k in range(NCORES):
        x_ext = np.concatenate([shift[k:k + 1], x[k]], axis=0).astype(nbf16)
        in_maps.append({
            "x_ext": np.ascontiguousarray(x_ext),
            "maa_k": mk, "maa_r": mr, "wrt": wrt,
            "wk": np.ascontiguousarray(wkb[EPC * k:EPC * (k + 1)]),
            "wv": np.ascontiguousarray(wvb[EPC * k:EPC * (k + 1)]),
            "shiftM": sm.astype(nbf16), "eM": em.astype(nbf16),
            **idxs[k],
        })
    return cfg, in_maps


def kernel(x, token_ids, shift_state, time_maa_k, time_maa_r,
           w_recept, w_key, w_value, _trace=False):
    cfg, in_maps = _prepare_inputs(x, token_ids, shift_state, time_maa_k,
                                   time_maa_r, w_recept, w_key, w_value)
    if cfg not in _CACHE:
        _CACHE[cfg] = _build_nc(cfg)
    nc = _CACHE[cfg]
    res = run_bass_kernel_spmd(nc, in_maps, core_ids=list(range(NCORES)),
                               trace=_trace)
    kernel.last_result = res
    y = np.stack([res.results[k]["out"] for k in range(NCORES)], axis=0)
    return y.astype(np.float32)


# revision 19
# speedup vs baseline: 1.0820x; 1.0820x over previous
"""Expert-parallel CMoE kernel for 8 Trainium2 NeuronCores.

Sharding (hardcoded for B=8, T=2048, D=1024, F=2048, E=16, C=1024):
  core k owns batch k (token shift, receptance, output) and experts
  {2k, 2k+1} (FFN). Hash routing is int math on token_ids, done on host;
  the resulting permutations ship to the cores as index tensors.

v4 schedule: x arrives bf16 and loads in four 1MB DMAs; the token shift
runs on the PE as dxprev = (S - I)@x + E@x_prev (exact in fp32 PSUM), so
phase A needs no second x stream; xr transposes on-chip via the XBAR
DMA; the receptance matmuls fill the PE while the two dispatch
all-to-alls fly; expert weights stream in 1MB groups just-in-time; the
combine is four balanced all-to-alls (expert x slot-parity) overlapped
with FFN compute; phase D runs in 512-token steps. DMA instruction
count is kept low throughout so the tile framework's DMA-completion
semaphores don't chain independent streams behind the collectives.
"""
import sys

for _p in ("/opt/trn_rl_repo", "/root/.axon_site/_ro/trn_rl_repo"):
    if _p not in sys.path:
        sys.path.append(_p)

import numpy as np
import ml_dtypes

import concourse.bass as bass
import concourse.bacc as bacc
import concourse.mybir as mybir
import concourse.tile as tile
from concourse.bass_utils import run_bass_kernel_spmd

P = 128
B, T, D, F, E = 8, 2048, 1024, 2048, 16
N = B * T
C = max(4, N // E)          # 1024
HASH_PRIME = 5099
NCORES = 8
EPC = E // NCORES           # experts per core = 2
DC = D // P                 # 8
FC = F // P                 # 16
TP = T // P                 # 16
HC = C // 2                 # 512, combine half size
BF16 = mybir.dt.bfloat16
F32 = mybir.dt.float32
I16 = mybir.dt.int16
I32 = mybir.dt.int32
nbf16 = ml_dtypes.bfloat16
AF = mybir.ActivationFunctionType

_CACHE = {}


def _r16(v):
    return int(-(-int(v) // 16) * 16)


def _wrap16(a):
    a = np.asarray(a, np.int16)
    w = a.reshape(-1, 16).T.copy()       # j at [j%16, j//16]
    return np.tile(w, (8, 1))            # replicated across 8 Q7 cores


def _route(token_ids):
    tid = np.asarray(token_ids).reshape(N).astype(np.int64)
    e = (tid * HASH_PRIME) % E
    onehot = (e[:, None] == np.arange(E)).astype(np.int64)
    pos = onehot.cumsum(0)[np.arange(N), e] - 1
    keep = pos < C
    return e, pos, keep


def _build_indices(token_ids):
    e, pos, keep = _route(token_ids)
    src = np.arange(N) // T
    dst = e // EPC
    el = e % EPC
    local_t = np.arange(N) % T
    # expert slots interleaved by arrival parity so each (src, dst) pair
    # contributes evenly to both combine halves
    slot = (pos % 2) * HC + pos // 2
    h = pos % 2

    def pack(mask):
        rank = np.zeros(N, np.int64)
        cnt = np.zeros((NCORES, NCORES), np.int64)
        for n in np.nonzero(mask)[0]:
            rank[n] = cnt[src[n], dst[n]]
            cnt[src[n], dst[n]] += 1
        return rank, _r16(cnt.max())

    # ---- dispatch: two chunks split by local token index
    in_a = keep & (local_t < T // 2)
    in_b = keep & (local_t >= T // 2)
    rank_a, Ka = pack(in_a)
    rank_b, Kb = pack(in_b)
    srcA = np.where(in_a, dst * Ka + rank_a, NCORES * Ka)
    srcB = np.where(in_b, dst * Kb + rank_b, NCORES * Kb)

    ZR1 = NCORES * (Ka + Kb)
    recv_row = np.full((NCORES, EPC * C), ZR1, np.int64)
    for n in np.nonzero(in_a)[0]:
        recv_row[dst[n], el[n] * C + slot[n]] = src[n] * Ka + rank_a[n]
    for n in np.nonzero(in_b)[0]:
        recv_row[dst[n], el[n] * C + slot[n]] = \
            NCORES * Ka + src[n] * Kb + rank_b[n]

    # ---- combine: four chunks (el, h)
    Kc = {}
    rank_c = np.zeros(N, np.int64)
    for eli in range(EPC):
        for hi in range(2):
            m = keep & (el == eli) & (h == hi)
            r_, K_ = pack(m)
            Kc[(eli, hi)] = K_
            rank_c[m] = r_[m]
    chunks = [(0, 0), (0, 1), (1, 0), (1, 1)]
    off = {}
    acc = 0
    for ch in chunks:
        off[ch] = acc
        acc += NCORES * Kc[ch]
    ZROW = acc

    s2 = np.zeros((NCORES, EPC, C), np.int64)
    for eli in range(EPC):
        for hi in range(2):
            s2[:, eli, hi * HC:(hi + 1) * HC] = NCORES * Kc[(eli, hi)]
    ygather = np.full(N, ZROW, np.int64)
    for n in np.nonzero(keep)[0]:
        ch = (el[n], h[n])
        s2[dst[n], el[n], slot[n]] = src[n] * Kc[ch] + rank_c[n]
        ygather[n] = off[ch] + dst[n] * Kc[ch] + rank_c[n]

    per_core = []
    for k in range(NCORES):
        tok = slice(k * T, (k + 1) * T)
        per_core.append({
            "srcA32": srcA[tok].astype(np.int32).reshape(TP, P).T.copy(),
            "srcB32": srcB[tok].astype(np.int32).reshape(TP, P).T.copy(),
            "slot16": _wrap16(recv_row[k]),
            "s232": np.ascontiguousarray(
                s2[k].reshape(EPC * C // P, P).T.astype(np.int32)),
            "ygather16": _wrap16(ygather[tok]),
        })
    cfg = (Ka, Kb, Kc[(0, 0)], Kc[(0, 1)], Kc[(1, 0)], Kc[(1, 1)])
    return cfg, per_core


def _build_nc(cfg):
    Ka, Kb, K00, K01, K10, K11 = cfg
    Kc = {(0, 0): K00, (0, 1): K01, (1, 0): K10, (1, 1): K11}
    chunks = [(0, 0), (0, 1), (1, 0), (1, 1)]
    off = {}
    acc = 0
    for ch in chunks:
        off[ch] = acc
        acc += NCORES * Kc[ch]
    R2 = acc
    ZR1 = NCORES * (Ka + Kb)
    rg = [list(range(NCORES))]

    nc = bacc.Bacc("TRN2", target_bir_lowering=False, debug=False,
                   num_devices=NCORES)

    x_ext = nc.dram_tensor("x_ext", [T + 1, D], BF16, kind="ExternalInput")
    maa_k = nc.dram_tensor("maa_k", [1, D], BF16, kind="ExternalInput")
    maa_r = nc.dram_tensor("maa_r", [1, D], BF16, kind="ExternalInput")
    wrt = nc.dram_tensor("wrt", [D, D], BF16, kind="ExternalInput")
    wk = nc.dram_tensor("wk", [EPC, D, F], BF16, kind="ExternalInput")
    wv = nc.dram_tensor("wv", [EPC, F, D], BF16, kind="ExternalInput")
    shiftM = nc.dram_tensor("shiftM", [P, P], BF16, kind="ExternalInput")
    eM = nc.dram_tensor("eM", [P, P], BF16, kind="ExternalInput")
    srcA32 = nc.dram_tensor("srcA32", [P, TP], I32, kind="ExternalInput")
    srcB32 = nc.dram_tensor("srcB32", [P, TP], I32, kind="ExternalInput")
    slot16 = nc.dram_tensor("slot16", [P, EPC * C // 16], I16,
                            kind="ExternalInput")
    s232 = nc.dram_tensor("s232", [P, EPC * C // P], I32,
                          kind="ExternalInput")
    ygather16 = nc.dram_tensor("ygather16", [P, T // 16], I16,
                               kind="ExternalInput")
    iota16 = nc.dram_tensor("iota16", [P, T // 16], I16,
                            kind="ExternalInput")
    out = nc.dram_tensor("out", [T, D], F32, kind="ExternalOutput")

    with tile.TileContext(nc) as tc:
        with (
            tc.tile_pool(name="dram", bufs=1, space="DRAM") as dram,
            tc.tile_pool(name="misc", bufs=1) as misc,
        ):
            a1a = dram.tile([NCORES * Ka + 1, D], BF16)
            a1b = dram.tile([NCORES * Kb + 1, D], BF16)
            recv1 = dram.tile([ZR1 + 1, D], BF16)
            a2 = {ch: dram.tile([NCORES * Kc[ch] + 1, D], BF16,
                                name=f"a2_{ch[0]}{ch[1]}")
                  for ch in chunks}
            recv2 = dram.tile([R2 + 1, D], BF16)
            r_dram = dram.tile([T, D], BF16)
            xr_dram = dram.tile([T, D], BF16)
            warm_in = dram.tile([NCORES, 64], BF16)
            warm_out = dram.tile([NCORES, 64], BF16)

            # warm-up all-to-all: absorbs core launch skew so the real
            # dispatch collectives pay no rendezvous wait
            wz = misc.tile([NCORES, 64], BF16)
            nc.vector.memzero(wz[:])
            nc.sync.dma_start(out=warm_in[:], in_=wz[:])
            nc.gpsimd.collective_compute(
                "AllToAll", mybir.AluOpType.bypass, replica_groups=rg,
                ins=[warm_in[:]], outs=[warm_out[:]])
            sm_sb = misc.tile([P, P], BF16)
            nc.sync.dma_start(out=sm_sb[:], in_=shiftM[:])
            em_sb = misc.tile([P, P], BF16)
            nc.sync.dma_start(out=em_sb[:], in_=eM[:])
            maakb = misc.tile([P, D], BF16)
            nc.sync.dma_start(out=maakb[:], in_=maa_k[:].to_broadcast([P, D]))
            maarb = misc.tile([P, D], BF16)
            nc.sync.dma_start(out=maarb[:], in_=maa_r[:].to_broadcast([P, D]))

            # index tensors ride the gpsimd SWDGE (its queue is idle early)
            sA32 = misc.tile([P, TP], I32)
            nc.gpsimd.dma_start(out=sA32[:], in_=srcA32[:])
            sB32 = misc.tile([P, TP], I32)
            nc.gpsimd.dma_start(out=sB32[:], in_=srcB32[:])
            sl16 = misc.tile([P, EPC * C // 16], I16)
            nc.gpsimd.dma_start(out=sl16[:], in_=slot16[:])
            s2sb = misc.tile([P, EPC * C // P], I32)
            nc.gpsimd.dma_start(out=s2sb[:], in_=s232[:])
            yg16 = misc.tile([P, T // 16], I16)
            nc.gpsimd.dma_start(out=yg16[:], in_=ygather16[:])
            io16 = misc.tile([P, T // 16], I16)
            nc.gpsimd.dma_start(out=io16[:], in_=iota16[:])

            zrow = misc.tile([1, D], BF16)
            nc.vector.memzero(zrow[:])

            wrt_sb = misc.tile([P, DC, D], BF16)
            nc.scalar.dma_start(out=wrt_sb[:],
                                in_=wrt.rearrange("(c p) e -> p c e", p=P))

            wk_t = {}
            wv_t = {}
            with (
                tc.tile_pool(name="pwk", bufs=4) as pwk,
                tc.tile_pool(name="pwv", bufs=4) as pwv,
            ):
                # ---- phase A: PE token shift + dispatch scatter; then the
                # receptance matmuls run while the dispatch A2As fly.
                with (
                    tc.tile_pool(name="pa", bufs=3) as pa,
                    tc.tile_pool(name="pax", bufs=1) as pax,
                    tc.tile_pool(name="prx", bufs=1) as prx,
                    tc.tile_pool(name="prs", bufs=2) as prs,
                    tc.tile_pool(name="psB", bufs=2, space="PSUM") as psB,
                ):
                    xch = [pax.tile([P, 4, D], BF16, name=f"xch{q}")
                           for q in range(4)]
                    for q in range(4):
                        nc.sync.dma_start(
                            out=xch[q][:],
                            in_=x_ext[1 + q * 512:1 + (q + 1) * 512, :]
                            .rearrange("(a p) d -> p a d", p=P))
                    xm1 = pax.tile([P, D], BF16, name="xm1")
                    nc.vector.memzero(xm1[:])
                    nc.sync.dma_start(out=xm1[P - 1:P, :], in_=x_ext[0:1, :])
                    nc.sync.dma_start(out=recv1[ZR1:ZR1 + 1, :], in_=zrow[:])
                    nc.sync.dma_start(out=recv2[R2:R2 + 1, :], in_=zrow[:])
                    xrT = [prx.tile([P, DC, 4 * P], BF16, name=f"xrT{g}")
                           for g in range(4)]
                    xrgs = []

                    def emit_xr_store(g, xrg):
                        nc.scalar.dma_start(
                            out=xr_dram[g * 512:(g + 1) * 512, :].rearrange(
                                "(a p) d -> p a d", p=P),
                            in_=xrg[:])

                    def emit_xrT_gather(g):
                        nc.gpsimd.dma_gather(
                            out_ap=xrT[g][:], in_ap=xr_dram[:],
                            idxs_ap=io16[:, g * 32:(g + 1) * 32],
                            num_idxs=512, num_idxs_reg=512, elem_size=D,
                            transpose=True)

                    def emit_recept_group(g):
                        rsb = prs.tile([P, 4, D], BF16, tag="rsb")
                        for u in range(4):
                            pr0 = psB.tile([P, 512], F32, space="PSUM",
                                           tag="pr0")
                            pr1 = psB.tile([P, 512], F32, space="PSUM",
                                           tag="pr1")
                            for dc in range(DC):
                                nc.tensor.matmul(
                                    out=pr0[:],
                                    lhsT=xrT[g][:, dc, u * P:(u + 1) * P],
                                    rhs=wrt_sb[:, dc, 0:512],
                                    start=(dc == 0), stop=(dc == DC - 1))
                                nc.tensor.matmul(
                                    out=pr1[:],
                                    lhsT=xrT[g][:, dc, u * P:(u + 1) * P],
                                    rhs=wrt_sb[:, dc, 512:1024],
                                    start=(dc == 0), stop=(dc == DC - 1))
                            nc.scalar.activation(out=rsb[:, u, 0:512],
                                                 in_=pr0[:], func=AF.Sigmoid)
                            nc.scalar.activation(out=rsb[:, u, 512:1024],
                                                 in_=pr1[:], func=AF.Sigmoid)
                        nc.scalar.dma_start(
                            out=r_dram[g * 512:(g + 1) * 512, :].rearrange(
                                "(a p) d -> p a d", p=P),
                            in_=rsb[:])

                    for t in range(TP):
                        xcb = xch[t // 4][:, t % 4, :]
                        prev = xm1[:] if t == 0 else \
                            xch[(t - 1) // 4][:, (t - 1) % 4, :]
                        psx0 = psB.tile([P, 512], F32, space="PSUM",
                                        tag="psx0")
                        psx1 = psB.tile([P, 512], F32, space="PSUM",
                                        tag="psx1")
                        # dxprev = (S - I) @ x_tile + E @ x_prev_tile
                        nc.tensor.matmul(out=psx0[:], lhsT=sm_sb[:],
                                         rhs=xcb[:, 0:512],
                                         start=True, stop=False)
                        nc.tensor.matmul(out=psx0[:], lhsT=em_sb[:],
                                         rhs=prev[:, 0:512],
                                         start=False, stop=True)
                        nc.tensor.matmul(out=psx1[:], lhsT=sm_sb[:],
                                         rhs=xcb[:, 512:1024],
                                         start=True, stop=False)
                        nc.tensor.matmul(out=psx1[:], lhsT=em_sb[:],
                                         rhs=prev[:, 512:1024],
                                         start=False, stop=True)
                        tk = pa.tile([P, D], BF16, tag="tk")
                        nc.vector.tensor_mul(out=tk[:, 0:512], in0=psx0[:],
                                             in1=maakb[:, 0:512])
                        nc.vector.tensor_mul(out=tk[:, 512:1024], in0=psx1[:],
                                             in1=maakb[:, 512:1024])
                        xk = pa.tile([P, D], BF16, tag="xk")
                        nc.vector.tensor_add(out=xk[:], in0=tk[:], in1=xcb)
                        if t < TP // 2:
                            nc.gpsimd.indirect_dma_start(
                                out=a1a[:],
                                out_offset=bass.IndirectOffsetOnAxis(
                                    ap=sA32[:, t:t + 1], axis=0),
                                in_=xk[:], in_offset=None)
                        else:
                            nc.gpsimd.indirect_dma_start(
                                out=a1b[:],
                                out_offset=bass.IndirectOffsetOnAxis(
                                    ap=sB32[:, t:t + 1], axis=0),
                                in_=xk[:], in_offset=None)
                        if t == TP // 2 - 1:
                            nc.gpsimd.collective_compute(
                                "AllToAll", mybir.AluOpType.bypass,
                                replica_groups=rg,
                                ins=[a1a[0:NCORES * Ka, :]],
                                outs=[recv1[0:NCORES * Ka, :]])
                        tr = pa.tile([P, D], BF16, tag="tr")
                        nc.vector.tensor_mul(out=tr[:, 0:512], in0=psx0[:],
                                             in1=maarb[:, 0:512])
                        nc.vector.tensor_mul(out=tr[:, 512:1024], in0=psx1[:],
                                             in1=maarb[:, 512:1024])
                        if t % 4 == 0:
                            xrg = prs.tile([P, 4, D], BF16, tag="xrg")
                            xrgs.append(xrg)
                        nc.vector.tensor_add(out=xrgs[t // 4][:, t % 4, :],
                                             in0=tr[:], in1=xcb)
                        if t % 4 == 3:
                            emit_xr_store(t // 4, xrgs[t // 4])
                            emit_xrT_gather(t // 4)
                        if t % 4 == 0:
                            q = t // 4
                            wk_t[(0, q)] = pwk.tile([P, DC, 512], BF16,
                                                    tag="wk", name=f"wk0_{q}")
                            nc.scalar.dma_start(
                                out=wk_t[(0, q)][:],
                                in_=wk[0].rearrange("(c p) f -> p c f", p=P)
                                [:, :, q * 512:(q + 1) * 512])
                    nc.gpsimd.collective_compute(
                        "AllToAll", mybir.AluOpType.bypass, replica_groups=rg,
                        ins=[a1b[0:NCORES * Kb, :]],
                        outs=[recv1[NCORES * Ka:ZR1, :]])
                    for g in range(4):
                        emit_recept_group(g)

                # ---------------- phase C: expert FFNs
                with (
                    tc.tile_pool(name="pfx", bufs=2) as pfx,
                    tc.tile_pool(name="pfh", bufs=2) as pfh,
                    tc.tile_pool(name="pfr", bufs=2) as pfr,
                    tc.tile_pool(name="pfy", bufs=3) as pfy,
                    tc.tile_pool(name="psH", bufs=2, space="PSUM") as psH,
                    tc.tile_pool(name="psY", bufs=2, space="PSUM") as psY,
                ):
                    XTs = []
                    for el in range(EPC):
                        pair = []
                        for hh in range(2):
                            XTh = pfx.tile([P, DC, 512], BF16, tag=f"XT{hh}",
                                           name=f"XT{el}_{hh}")
                            col0 = el * (C // 16) + hh * 32
                            nc.gpsimd.dma_gather(
                                out_ap=XTh[:], in_ap=recv1[:],
                                idxs_ap=sl16[:, col0:col0 + 32],
                                num_idxs=512, num_idxs_reg=512, elem_size=D,
                                transpose=True)
                            pair.append(XTh)
                        XTs.append(pair)
                    for el in range(EPC):
                        XT0, XT1 = XTs[el]
                        ht = [pfh.tile([P, 4, C], BF16, tag=f"ht{g}",
                                       name=f"ht{el}_{g}")
                              for g in range(4)]
                        for ft in range(FC):
                            wkt = wk_t[(el, ft // 4)]
                            wcol = (ft % 4) * P
                            ph0 = psH.tile([P, 512], F32, space="PSUM",
                                           tag="ph0")
                            ph1 = psH.tile([P, 512], F32, space="PSUM",
                                           tag="ph1")
                            for dc in range(DC):
                                nc.tensor.matmul(
                                    out=ph0[:],
                                    lhsT=wkt[:, dc, wcol:wcol + P],
                                    rhs=XT0[:, dc, :],
                                    start=(dc == 0), stop=(dc == DC - 1))
                                nc.tensor.matmul(
                                    out=ph1[:],
                                    lhsT=wkt[:, dc, wcol:wcol + P],
                                    rhs=XT1[:, dc, :],
                                    start=(dc == 0), stop=(dc == DC - 1))
                            hr0 = pfr.tile([P, 512], BF16, tag="hr0")
                            nc.scalar.activation(out=hr0[:], in_=ph0[:],
                                                 func=AF.Relu)
                            nc.vector.tensor_mul(
                                out=ht[ft // 4][:, ft % 4, 0:512],
                                in0=hr0[:], in1=hr0[:])
                            hr1 = pfr.tile([P, 512], BF16, tag="hr1")
                            nc.scalar.activation(out=hr1[:], in_=ph1[:],
                                                 func=AF.Relu)
                            nc.vector.tensor_mul(
                                out=ht[ft // 4][:, ft % 4, 512:1024],
                                in0=hr1[:], in1=hr1[:])
                            if ft % 4 == 0:
                                # JIT weight prefetch on the scalar queue:
                                # wv for this expert's FFN2, wk for the next
                                # expert (after slot (0, q) frees)
                                q = ft // 4
                                wv_t[(el, q)] = pwv.tile(
                                    [P, 4, D], BF16, tag="wv",
                                    name=f"wv{el}_{q}")
                                nc.scalar.dma_start(
                                    out=wv_t[(el, q)][:],
                                    in_=wv[el][q * 512:(q + 1) * 512, :]
                                    .rearrange("(a p) d -> p a d", p=P))
                            if el == 0 and ft % 4 == 3:
                                q = ft // 4
                                wk_t[(1, q)] = pwk.tile([P, DC, 512], BF16,
                                                        tag="wk",
                                                        name=f"wk1_{q}")
                                nc.scalar.dma_start(
                                    out=wk_t[(1, q)][:],
                                    in_=wk[1].rearrange("(c p) f -> p c f",
                                                        p=P)
                                    [:, :, q * 512:(q + 1) * 512])
                        for tt in range(C // P):
                            py0 = psY.tile([P, 512], F32, space="PSUM",
                                           tag="py0")
                            py1 = psY.tile([P, 512], F32, space="PSUM",
                                           tag="py1")
                            for fc in range(FC):
                                wvt = wv_t[(el, fc // 4)]
                                nc.tensor.matmul(
                                    out=py0[:],
                                    lhsT=ht[fc // 4][:, fc % 4,
                                                     tt * P:(tt + 1) * P],
                                    rhs=wvt[:, fc % 4, 0:512],
                                    start=(fc == 0), stop=(fc == FC - 1))
                                nc.tensor.matmul(
                                    out=py1[:],
                                    lhsT=ht[fc // 4][:, fc % 4,
                                                     tt * P:(tt + 1) * P],
                                    rhs=wvt[:, fc % 4, 512:1024],
                                    start=(fc == 0), stop=(fc == FC - 1))
                            ysb = pfy.tile([P, D], BF16, tag="ysb")
                            nc.vector.tensor_copy(out=ysb[:, 0:512],
                                                  in_=py0[:])
                            nc.vector.tensor_copy(out=ysb[:, 512:1024],
                                                  in_=py1[:])
                            scol = el * (C // P) + tt
                            ch = (el, tt // 4)
                            nc.gpsimd.indirect_dma_start(
                                out=a2[ch][:],
                                out_offset=bass.IndirectOffsetOnAxis(
                                    ap=s2sb[:, scol:scol + 1], axis=0),
                                in_=ysb[:], in_offset=None)
                            if tt == 3 or tt == C // P - 1:
                                nc.gpsimd.collective_compute(
                                    "AllToAll", mybir.AluOpType.bypass,
                                    replica_groups=rg,
                                    ins=[a2[ch][0:NCORES * Kc[ch], :]],
                                    outs=[recv2[off[ch]:
                                                off[ch] + NCORES * Kc[ch], :]])

                # ---------------- phase D: gather own rows, multiply by r
                with tc.tile_pool(name="pd", bufs=3) as pd:
                    for st in range(T // 512):
                        yg = pd.tile([P, 4, D], BF16, tag="yg")
                        nc.gpsimd.dma_gather(
                            out_ap=yg[:], in_ap=recv2[:],
                            idxs_ap=yg16[:, st * 32:(st + 1) * 32],
                            num_idxs=512, num_idxs_reg=512, elem_size=D,
                            transpose=False)
                        rw = pd.tile([P, 4, D], BF16, tag="rw")
                        nc.sync.dma_start(
                            out=rw[:],
                            in_=r_dram[st * 512:(st + 1) * 512, :].rearrange(
                                "(a p) d -> p a d", p=P))
                        yo = pd.tile([P, 4, D], F32, tag="yo")
                        nc.vector.tensor_mul(out=yo[:], in0=yg[:], in1=rw[:])
                        nc.scalar.dma_start(
                            out=out[st * 512:(st + 1) * 512, :].rearrange(
                                "(a p) d -> p a d", p=P),
                            in_=yo[:])

    nc.finalize()
    return nc


def _prepare_inputs(x, token_ids, shift_state, time_maa_k, time_maa_r,
                    w_recept, w_key, w_value):
    cfg, idxs = _build_indices(token_ids)
    x = np.asarray(x, np.float32)
    shift = np.asarray(shift_state, np.float32)
    wrt = np.ascontiguousarray(np.asarray(w_recept, np.float32).T).astype(nbf16)
    wkb = np.asarray(w_key, np.float32).astype(nbf16)
    wvb = np.asarray(w_value, np.float32).astype(nbf16)
    mk = np.asarray(time_maa_k, np.float32)[None, :].astype(nbf16)
    mr = np.asarray(time_maa_r, np.float32)[None, :].astype(nbf16)
    # token-shift matrices: dxprev = (S - I) @ x_tile + E @ x_prev_tile
    sm = np.zeros((P, P), np.float32)
    for j in range(P):
        sm[j, j] = -1.0
        if j >= 1:
            sm[j - 1, j] = 1.0
    em = np.zeros((P, P), np.float32)
    em[P - 1, 0] = 1.0
    iota = np.tile(np.arange(T, dtype=np.int16).reshape(-1, 16).T, (8, 1))

    in_maps = []
    for k in range(NCORES):
        x_ext = np.concatenate([shift[k:k + 1], x[k]], axis=0).astype(nbf16)
        in_maps.append({
            "x_ext": np.ascontiguousarray(x_ext),
            "maa_k": mk, "maa_r": mr, "wrt": wrt,
            "wk": np.ascontiguousarray(wkb[EPC * k:EPC * (k + 1)]),
            "wv": np.ascontiguousarray(wvb[EPC * k:EPC * (k + 1)]),
            "shiftM": sm.astype(nbf16), "eM": em.astype(nbf16),
            "iota16": iota,
            **idxs[k],
        })
    return cfg, in_maps


def kernel(x, token_ids, shift_state, time_maa_k, time_maa_r,
           w_recept, w_key, w_value, _trace=False):
    cfg, in_maps = _prepare_inputs(x, token_ids, shift_state, time_maa_k,
                                   time_maa_r, w_recept, w_key, w_value)
    if cfg not in _CACHE:
        _CACHE[cfg] = _build_nc(cfg)
    nc = _CACHE[cfg]
    res = run_bass_kernel_spmd(nc, in_maps, core_ids=list(range(NCORES)),
                               trace=_trace)
    kernel.last_result = res
    y = np.stack([res.results[k]["out"] for k in range(NCORES)], axis=0)
    return y.astype(np.float32)
